# revision 8
# baseline (speedup 1.0000x reference)
"""Cosine-similarity kernel (x[16384,512] vs weights[4096,512] -> [16384,4096])
on 8 Trainium2 NeuronCores, data-parallel over the x batch dim.

Per core: x shard [2048,512] fp32, full weights [4096,512] fp32.
  out = normalize(x) @ normalize(w).T
Implemented as: raw x -> PE transpose -> f32r; w -> row-normalize -> PE
transpose -> f32r; f32r matmuls (1 cyc/row) accumulate K=512 in PSUM;
x-row 1/norm applied as ACT scale at PSUM eviction.
"""
import numpy as np

B, D, N = 16384, 512, 4096
NCORES = 8
BS = B // NCORES          # 2048 rows per core
MT = BS // 128            # 16 m-tiles
NT = N // 128             # 32 w row-tiles
KC = D // 128             # 4 k-chunks
NB = N // 512             # 8 n-blocks of 512

_cached = {}


def _build():
    import concourse.bass as bass
    import concourse.mybir as mybir
    import concourse.tile as tile
    from concourse import bacc
    from concourse.masks import make_identity

    F32, F32R = mybir.dt.float32, mybir.dt.float32r
    nc = bacc.Bacc(None, target_bir_lowering=False)
    x = nc.dram_tensor("x", [BS, D], F32, kind="ExternalInput")
    w = nc.dram_tensor("weights", [N, D], F32, kind="ExternalInput")
    o = nc.dram_tensor("out", [BS, N], F32, kind="ExternalOutput")

    with tile.TileContext(nc) as tc:
        with (
            tc.tile_pool(name="const", bufs=1) as const,
            tc.tile_pool(name="big", bufs=1) as big,
            tc.tile_pool(name="stage", bufs=6) as stage,
            tc.tile_pool(name="ostage", bufs=8) as ostage,
            tc.tile_pool(name="mmps", bufs=4, space="PSUM") as mmps,
            tc.tile_pool(name="trps", bufs=4, space="PSUM") as trps,
        ):
            ident = const.tile([128, 128], F32, name="ident")
            make_identity(nc, ident[:])
            rx = const.tile([128, MT], F32, name="rx")

            wT = [big.tile([128, N], F32R, name=f"wT{k}") for k in range(KC)]
            xT = [big.tile([128, BS], F32R, name=f"xT{k}") for k in range(KC)]

            def w_prep(j):
                wt = stage.tile([128, D], F32, name="wt", tag="wt")
                nc.sync.dma_start(wt[:], w[j * 128:(j + 1) * 128, :])
                sq = stage.tile([128, D], F32, name="sq", tag="sq")
                ss = stage.tile([128, 1], F32, name="ss", tag="ss")
                nc.scalar.activation(
                    sq[:], wt[:], mybir.ActivationFunctionType.Square,
                    accum_out=ss[:])
                inv = stage.tile([128, 1], F32, name="inv", tag="inv")
                nc.vector.reciprocal(inv[:], ss[:])
                rw = stage.tile([128, 1], F32, name="rw", tag="rw")
                nc.scalar.sqrt(rw[:], inv[:])
                wn = stage.tile([128, D], F32, name="wn", tag="wn")
                nc.scalar.mul(wn[:], wt[:], rw[:])
                for k in range(KC):
                    pt = trps.tile([128, 128], F32, name="pt", tag="pt")
                    nc.tensor.transpose(pt[:], wn[:, k * 128:(k + 1) * 128], ident[:])
                    nc.vector.tensor_copy(wT[k][:, j * 128:(j + 1) * 128], pt[:])

            # ---- x: load, compute 1/norm, transpose raw to [K, BS] f32r ----
            for m in range(MT):
                xt = stage.tile([128, D], F32, name="xt", tag="wt")
                nc.sync.dma_start(xt[:], x[m * 128:(m + 1) * 128, :])
                sq = stage.tile([128, D], F32, name="sqx", tag="sq")
                ss = stage.tile([128, 1], F32, name="ssx", tag="ss")
                nc.scalar.activation(
                    sq[:], xt[:], mybir.ActivationFunctionType.Square,
                    accum_out=ss[:])
                inv = stage.tile([128, 1], F32, name="invx", tag="inv")
                nc.vector.reciprocal(inv[:], ss[:])
                nc.scalar.sqrt(rx[:, m:m + 1], inv[:])
                for k in range(KC):
                    pt = trps.tile([128, 128], F32, name="ptx", tag="pt")
                    nc.tensor.transpose(pt[:], xt[:, k * 128:(k + 1) * 128], ident[:])
                    nc.vector.tensor_copy(xT[k][:, m * 128:(m + 1) * 128], pt[:])

            # ---- main GEMM, pipelined with w-prep per n-block column ----
            for nb in range(NB):
                for j in range(nb * 4, nb * 4 + 4):
                    w_prep(j)
                for m in range(MT):
                    pm = mmps.tile([128, 512], F32, name="pm", tag="pm")
                    for k in range(KC):
                        nc.tensor.matmul(
                            pm[:],
                            xT[k][:, m * 128:(m + 1) * 128],
                            wT[k][:, nb * 512:(nb + 1) * 512],
                            start=(k == 0), stop=(k == KC - 1))
                    ot = ostage.tile([128, 512], F32, name="ot", tag="ot")
                    nc.scalar.mul(ot[:], pm[:], rx[:, m:m + 1])
                    nc.sync.dma_start(
                        o[m * 128:(m + 1) * 128, nb * 512:(nb + 1) * 512], ot[:])
    nc.compile()
    return nc


def kernel(x: np.ndarray, weights: np.ndarray) -> np.ndarray:
    from concourse.bass_utils import run_bass_kernel_spmd

    if "nc" not in _cached:
        _cached["nc"] = _build()
    nc = _cached["nc"]

    x = np.ascontiguousarray(x, dtype=np.float32)
    weights = np.ascontiguousarray(weights, dtype=np.float32)
    in_maps = [
        {"x": x[i * BS:(i + 1) * BS], "weights": weights} for i in range(NCORES)
    ]
    res = run_bass_kernel_spmd(nc, in_maps, list(range(NCORES)))
    return np.concatenate([res.results[i]["out"] for i in range(NCORES)], axis=0)
